# revision 46
# baseline (speedup 1.0000x reference)
"""Trainium2 Bass kernel for nn_BoostEnhancedAttention.

Reference computation:
    v   = (values @ W_v.T + b_v)                      # [B, NK, H*D_V]
    att = softmax(att3 * att12 interleaved, axis=k)   # [B, H, NQ, NK]
    out = (att @ v_per_head) @ W_o.T + b_o            # [B, NQ, D_MODEL]

Restructuring used here (verified vs reference, rel err ~1.2e-2):
  - Scores factor as s[b,h,q,k] = att3[b,h,q,c(k)] * att12[b,h,...f(k)];
    E = exp(s) built by DVE broadcast-multiply + ACT exp.
  - Attention applied BEFORE the projections: G[d_in,(h,q)] =
    sum_k values[k,d_in] * (E[k,(h,q)]-1) via fp8 e4m3 DoubleRow matmuls
    (ALL 32 k-tiles), plus exact fp32 ones-correction C = sum_k values
    folded into the ga-copy bias. expm1 shift halves fp8 quant error.
  - Softmax denominator Z by 4th-order Taylor: Z[h,q] =
    sum_c sum_j T_j[h,c]*a3[q,c]^j, T_j = sum_f a12^j/j! (host),
    evaluated by a short Horner chain on DVE + ones-matmuls. This
    replaces the per-group E-summation entirely (saves ~4.5us DVE per
    batch and removes the exp->sum dependency chain).
  - Projections after normalize, per head: U = ga @ W_v_h.T, then
    out = U.T-contraction with W_o + bias via K=1 matmul.
  - fp8 converts (E-1 -> e4m3) round-robined over DVE/ACT/Pool so no
    single elementwise engine bottlenecks; exp stays the ACT floor.

Sharding: data-parallel over batch, B=32 over 8 cores -> 4 batches/core.
No collectives needed; outputs concatenated on host.
"""

import numpy as np
import ml_dtypes

B, CH, CW, H, FH, FW = 32, 16, 16, 8, 4, 4
NQ = 64
NCELL = CH * CW          # 256 coarse cells (c)
F = FH * FW              # 16 fine positions per cell
NK = NCELL * F           # 4096
D_IN, D_V, D_MODEL = 512, 64, 512
N_CORES = 8
B_LOC = B // N_CORES     # 4
N_DT = 4                 # d_in tiles of 128
HQ = H * NQ              # 512
N_PAIR = H // 2          # head-pairs for the U projection tiles
NJ = 3                   # fitted quadratic for Z: coeffs T2..T0 per (c,h)
A12W = F * H * 2         # att12 pair-dup block width
TW = NJ * HQ             # T coeffs pre-expanded over q (packed DVE 2x APs)

BF16 = ml_dtypes.bfloat16
FP8 = ml_dtypes.float8_e4m3


def _k_perm():
    """perm[k'] -> original k, where k' = (half*16+f)*128 + c_loc."""
    perm = np.zeros(NK, np.int64)
    c = np.arange(NCELL)
    ch_i, cw_i = c // CW, c % CW
    for half in range(2):
        for f in range(F):
            kt = half * F + f
            fh, fw = f // FW, f % FW
            cc = half * 128 + np.arange(128)
            perm[kt * 128:(kt + 1) * 128] = (
                ch_i[cc] * (FH * CW * FW) + fh * (CW * FW) + cw_i[cc] * FW + fw
            )
    return perm


_PERM = _k_perm()
_NC_CACHE = {}

# per-batch group schedule: (half, f0, FQ); b0 splits half0's first tiles.
# group (half=0, f0<4) stays bf16 (k-tiles 0-3): no fp8 convert, and the
# extra PE time keeps the HAM clock gate from throttling the ramp.
_ORDER_B0 = [(0, 0, 1), (0, 1, 1), (0, 2, 2), (0, 4, 4), (1, 0, 4),
             (0, 8, 4), (1, 4, 4), (0, 12, 4), (1, 8, 4), (1, 12, 4)]
_ORDER = [(0, 0, 4), (0, 4, 4), (1, 0, 4), (0, 8, 4),
          (1, 4, 4), (0, 12, 4), (1, 8, 4), (1, 12, 4)]
# engine for the fp8 convert of each fp8 group in order: D=DVE, A=ACT.
# A-convs serialize behind exps on the ACT queue, so keep them few and
# mid-batch (early ones stall the whole exp stream, late ones gate the
# last AV groups).
_CONV = "DDADADD"
# engine for the 4 ga copies (psum->sbuf + ones-correction bias)
_GA = "ADAD"


def _build_nc():
    from contextlib import ExitStack

    import concourse.bass as bass
    import concourse.tile as tile
    from concourse import bacc, mybir

    f32 = mybir.dt.float32
    bf16 = mybir.dt.bfloat16
    fp8 = mybir.dt.float8e4

    nc = bacc.Bacc("TRN2", target_bir_lowering=False, debug=False,
                   num_devices=N_CORES)

    values_dr = nc.dram_tensor("values_dr", [B_LOC, 7, 128, 4 * D_IN],
                               fp8, kind="ExternalInput")
    values_bf = nc.dram_tensor("values_bf", [B_LOC, 128, 8 * D_IN],
                               bf16, kind="ExternalInput")
    c_all = nc.dram_tensor("c_all", [B_LOC, 128, N_DT], f32,
                           kind="ExternalInput")
    att3_t = nc.dram_tensor("att3_t", [B_LOC, NCELL, HQ], bf16,
                            kind="ExternalInput")
    att12_pt = nc.dram_tensor("att12_pt", [B_LOC, NCELL, A12W + TW], bf16,
                              kind="ExternalInput")
    wv_all = nc.dram_tensor("wv_all", [128, N_DT * H * D_V], bf16,
                            kind="ExternalInput")
    wo_all = nc.dram_tensor("wo_all", [128, N_PAIR * D_MODEL], bf16,
                            kind="ExternalInput")
    beff = nc.dram_tensor("beff", [1, D_MODEL], bf16, kind="ExternalInput")
    out = nc.dram_tensor("out", [B_LOC * NQ, D_MODEL], f32,
                         kind="ExternalOutput")

    with tile.TileContext(nc) as tc, ExitStack() as ctx:
        const_pool = ctx.enter_context(tc.tile_pool(name="const", bufs=1))
        a3_pool = ctx.enter_context(tc.tile_pool(name="a3", bufs=2))
        a12r_pool = ctx.enter_context(tc.tile_pool(name="a12r", bufs=2))
        sc_pool = ctx.enter_context(tc.tile_pool(name="sc", bufs=6))
        et_pool = ctx.enter_context(tc.tile_pool(name="et", bufs=6))
        et8_pool = ctx.enter_context(tc.tile_pool(name="et8", bufs=7))
        vdr_pool = ctx.enter_context(tc.tile_pool(name="vdr", bufs=8))
        vbf_pool = ctx.enter_context(tc.tile_pool(name="vbf", bufs=2))
        p_pool = ctx.enter_context(tc.tile_pool(name="pp", bufs=2))
        zb_pool = ctx.enter_context(tc.tile_pool(name="zb", bufs=2))
        ga_pool = ctx.enter_context(tc.tile_pool(name="ga", bufs=2))
        usb_pool = ctx.enter_context(tc.tile_pool(name="usb", bufs=2))
        g_pool = ctx.enter_context(tc.tile_pool(name="gps", bufs=1, space="PSUM"))
        u_pool = ctx.enter_context(tc.tile_pool(name="ups", bufs=1, space="PSUM"))
        z_pool = ctx.enter_context(tc.tile_pool(name="zps", bufs=1, space="PSUM"))

        Q2 = NQ // 2

        def prologue(b, first=False):
            """Input DMAs + first score group for batch b."""
            a3_t = [a3_pool.tile([128, HQ], bf16, tag=f"a3_{hf}",
                                 name=f"a3_{b}_{hf}") for hf in range(2)]
            a12r_t = [a12r_pool.tile([128, A12W + TW], bf16, tag=f"a12r_{hf}",
                                     name=f"a12r_{b}_{hf}") for hf in range(2)]
            vbf = vbf_pool.tile([128, 8 * D_IN], bf16, tag="vbf",
                                name=f"vbf_{b}")
            if first:
                # startup: interleave so the critical chain (a3/a12 hf0 ->
                # mul -> exp, bf16 values for the first matmuls) is
                # dispatched first on the serial sync queue. b0 runs TWO
                # bf16 groups (8 k-tiles) as PE runway while the clock and
                # the score pipeline ramp up.
                nc.sync.dma_start(a3_t[0][:], att3_t.ap()[b, 0:128, :])
                nc.sync.dma_start(a12r_t[0][:], att12_pt.ap()[b, 0:128, :])
                nc.sync.dma_start(vbf[:], values_bf.ap()[b])
                nc.sync.dma_start(a3_t[1][:], att3_t.ap()[b, 128:256, :])
                nc.sync.dma_start(a12r_t[1][:], att12_pt.ap()[b, 128:256, :])
                vdr_pre = None
            else:
                for hf in range(2):
                    nc.sync.dma_start(a3_t[hf][:],
                                      att3_t.ap()[b, hf * 128:(hf + 1) * 128, :])
                    nc.sync.dma_start(a12r_t[hf][:],
                                      att12_pt.ap()[b, hf * 128:(hf + 1) * 128, :])
                nc.sync.dma_start(vbf[:, :4 * D_IN],
                                  values_bf.ap()[b][:, :4 * D_IN])
                vdr_pre = vdr_pool.tile([128, 4 * D_IN], fp8, tag="vdr",
                                        name=f"vdr_{b}_0")
                nc.sync.dma_start(vdr_pre[:], values_dr.ap()[b, 0])
            order = _ORDER_B0 if b == 0 else _ORDER
            # emit group 0 (mul+exp) plus group 1's mul ahead of the
            # previous epilogue so the score pipeline leads the PE
            et8s = {0: emit_group(b, order[0], None, a3_t, a12r_t)}
            sc1 = emit_mul(b, order[1], a3_t, a12r_t)
            return a3_t, a12r_t, vbf, order, et8s, vdr_pre, sc1

        def emit_mul(b, grp, a3_t, a12r_t):
            """Broadcast multiply for one score group -> sc tile."""
            half, f0, FQ = grp
            a3b = a3_t[half][:]
            in0 = bass.AP(a3b.tensor, a3b.offset,
                          [a3b.ap[0], [0, FQ], [NQ, H], [2, Q2], [1, 2]])
            sc = sc_pool.tile([128, 4 * HQ], bf16, tag="sc",
                              name=f"sc_{b}_{half}_{f0}")
            scb = sc[:]
            out_ap = bass.AP(scb.tensor, scb.offset,
                             [scb.ap[0], [HQ, FQ], [NQ, H], [2, Q2], [1, 2]])
            a12b = a12r_t[half][:]
            in1 = bass.AP(a12b.tensor, a12b.offset + f0 * H * 2,
                          [a12b.ap[0], [H * 2, FQ], [2, H], [0, Q2], [1, 2]])
            nc.vector.tensor_mul(out_ap, in0, in1)
            return sc

        def emit_group(b, grp, conv_eng, a3_t, a12r_t, sc=None):
            """One score group: broadcast multiply, exp, fp8 convert.
            conv_eng None -> bf16 group (no convert), returns et."""
            half, f0, FQ = grp
            if sc is None:
                sc = emit_mul(b, grp, a3_t, a12r_t)
            et = et_pool.tile([128, 4 * HQ], bf16, tag="et",
                              name=f"et_{b}_{half}_{f0}")
            nc.scalar.activation(et[:, :FQ * HQ], sc[:, :FQ * HQ],
                                 mybir.ActivationFunctionType.Exp)
            if conv_eng is None:
                return et
            et8 = et8_pool.tile([128, 4 * HQ], fp8, tag="et8",
                                name=f"et8_{b}_{half}_{f0}")
            n = FQ * HQ
            if conv_eng == "D":
                nc.vector.tensor_scalar_sub(et8[:, :n], et[:, :n], 1.0)
            else:
                nc.scalar.activation(et8[:, :n], et[:, :n],
                                     mybir.ActivationFunctionType.Identity,
                                     bias=neg1_sb[:])
            return et8

        def emit_poly(b, half, a3_t, a12r_t):
            """Z-poly for one half: P = (T2*a + T1)*a + T0, least-squares
            fitted quadratic coefficients (host). NOTE: keep off the Pool
            engine -- GpSimd shares SBUF ports with the DVE and slows every
            concurrent DVE op by ~1.65x (measured)."""
            P = p_pool.tile([128, HQ], bf16, tag=f"p_{half}",
                            name=f"p_{b}_{half}")
            a3b = a3_t[half][:]
            a12b = a12r_t[half][:]
            Pb = P[:]

            def tb(jj):
                # T coeffs pre-expanded over q on host: plain packed APs
                # keep the DVE in its 2x mode (broadcast APs drop to 1x)
                return bass.AP(a12b.tensor, a12b.offset + A12W + jj * HQ,
                               [a12b.ap[0], [1, HQ]])

            nc.vector.tensor_mul(Pb, a3b, tb(0))
            nc.vector.tensor_add(Pb, Pb, tb(1))
            nc.vector.tensor_mul(Pb, Pb, a3b)
            nc.vector.tensor_add(Pb, Pb, tb(2))
            return P

        ones_sb = const_pool.tile([128, 128], bf16)
        nc.vector.memset(ones_sb[:], 1.0)
        warm_sb = const_pool.tile([128, D_MODEL], bf16, name="warm_sb")
        nc.vector.memset(warm_sb[:], 1.0)
        # warm matmuls bridge engine boot -> first attention matmul so the
        # HAM clock gate ramps the PE before real work arrives
        warm = z_pool.tile([128, HQ], f32, tag="z", name="warm")
        for wi in range(12):
            nc.tensor.matmul(warm[:], ones_sb[:], warm_sb[:],
                             start=True, stop=True)
        beff_sb = const_pool.tile([1, D_MODEL], bf16)
        neg1_sb = const_pool.tile([128, 1], f32, name="neg1")
        nc.vector.memset(neg1_sb[:], -1.0)
        # dummy ACT op at t=0 hoists the exp table load off the critical path
        actwarm = const_pool.tile([128, 1], bf16, name="actwarm")
        nc.scalar.activation(actwarm[:], ones_sb[:, 0:1],
                             mybir.ActivationFunctionType.Exp)
        c_sb = const_pool.tile([128, B_LOC * N_DT], f32, name="c_sb")

        pro = prologue(0, first=True)

        for b in range(B_LOC):
            a3_t, a12r_t, vbf, order, et8s, vdr_pre, sc1 = pro
            vdr_t = {} if vdr_pre is None else {0: vdr_pre}
            nbf = 8 if b == 0 else 4
            gps = [g_pool.tile([128, HQ], f32, tag=f"g{dt}", name=f"g_{b}_{dt}",
                               bufs=(2 if dt < 2 else 1))
                   for dt in range(N_DT)]
            started = [False] * N_DT
            n_grp = len(order)
            P_t = [None, None]
            fp8_idx = 0

            for gi, grp in enumerate(order):
                half, f0, FQ = grp
                is_bf = (half == 0 and f0 < nbf)
                if gi in et8s:
                    et8 = et8s[gi]
                elif is_bf:
                    et8 = emit_group(b, grp, None, a3_t, a12r_t,
                                     sc=(sc1 if gi == 1 else None))
                else:
                    et8 = emit_group(b, grp, _CONV[fp8_idx], a3_t, a12r_t,
                                     sc=(sc1 if gi == 1 else None))
                if not is_bf:
                    fp8_idx += 1
                last_grp = (gi == n_grp - 1)
                eb = et8[:]
                if is_bf:
                    # bf16 AV: plain matmuls straight off the exp output
                    vb = vbf[:]
                    for j in range(FQ):
                        kt = f0 + j
                        for dt in range(N_DT):
                            nc.tensor.matmul(
                                gps[dt][:],
                                vbf[:, kt * D_IN + dt * 128:
                                    kt * D_IN + (dt + 1) * 128],
                                et8[:, j * HQ:(j + 1) * HQ],
                                start=not started[dt], stop=False)
                            started[dt] = True
                else:
                    vg = 4 * half + f0 // 4 - 1
                    if vg not in vdr_t:
                        vdr_t[vg] = vdr_pool.tile([128, 4 * D_IN], fp8,
                                                  tag="vdr",
                                                  name=f"vdr_{b}_{vg}")
                        nc.sync.dma_start(vdr_t[vg][:], values_dr.ap()[b, vg])
                    vdr = vdr_t[vg]
                    vb = vdr[:]
                    for pidx in range(2):
                        last_p = last_grp and pidx == 1
                        for dt in range(N_DT):
                            lhsT = bass.AP(vb.tensor,
                                           vb.offset + pidx * 2 * D_IN
                                           + dt * 128,
                                           [vb.ap[0], [D_IN, 2], [1, 128]])
                            rhs = bass.AP(eb.tensor, eb.offset + pidx * 2 * HQ,
                                          [eb.ap[0], [HQ, 2], [1, HQ]])
                            nc.tensor.matmul(
                                gps[dt][:], lhsT, rhs,
                                start=not started[dt], stop=last_p,
                                perf_mode=mybir.MatmulPerfMode.DoubleRow)
                            started[dt] = True

                # Z-poly per half on the DVE, early in the batch
                if gi == 0:
                    P_t[0] = emit_poly(b, 0, a3_t, a12r_t)
                elif gi == 1:
                    P_t[1] = emit_poly(b, 1, a3_t, a12r_t)
                if b == 0 and gi == 5:
                    # epilogue-only constants after the critical prefetches
                    nc.sync.dma_start(beff_sb[:], beff.ap())
                    for cb in range(B_LOC):
                        nc.sync.dma_start(c_sb[:, cb * N_DT:(cb + 1) * N_DT],
                                          c_all.ap()[cb])
                if b == 0 and gi == 6:
                    # projection weights mid-stream: transfer never contends
                    # with critical prefetches
                    wv_sb = const_pool.tile([128, N_DT * H * D_V], bf16,
                                            name="wv_sb")
                    nc.sync.dma_start(wv_sb[:], wv_all.ap())
                    wo_sb = const_pool.tile([128, N_PAIR * D_MODEL], bf16,
                                            name="wo_sb")
                    nc.sync.dma_start(wo_sb[:], wo_all.ap())

            if b + 1 < B_LOC:
                pro = prologue(b + 1)

            # Z = column sums of P via two ones-matmuls per half; output
            # partitions 0-63 get even-head sums, 64-127 odd heads
            zps = z_pool.tile([128, 2 * N_PAIR * NQ], f32, tag="z",
                              name=f"z_{b}")
            for hf in range(2):
                pb = P_t[hf][:]
                for hl in range(2):
                    mov = bass.AP(pb.tensor, pb.offset + hl * NQ,
                                  [pb.ap[0], [2 * NQ, N_PAIR], [1, NQ]])
                    nc.tensor.matmul(
                        zps[hl * 64:(hl + 1) * 64, 0:N_PAIR * NQ],
                        ones_sb[:, hl * 64:(hl + 1) * 64], mov,
                        start=(hf == 0), stop=(hf == 1))
            zbu = zb_pool.tile([128, N_PAIR * NQ], f32)
            nc.vector.reciprocal_approx_fast(zbu[:], zps[:, 0:N_PAIR * NQ])

            # unnormalized attention output to sbuf; bias adds the exact
            # fp32 sum of all values rows (the "+1" of each expm1 key)
            ga = ga_pool.tile([128, N_DT * HQ], bf16, tag="ga", name=f"ga_{b}")
            for dt in range(N_DT):
                cb = c_sb[:, b * N_DT + dt:b * N_DT + dt + 1]
                if _GA[dt] == "A":
                    nc.scalar.activation(
                        ga[:, dt * HQ:(dt + 1) * HQ], gps[dt][:],
                        mybir.ActivationFunctionType.Identity, bias=cb)
                else:
                    nc.vector.tensor_scalar_add(
                        ga[:, dt * HQ:(dt + 1) * HQ], gps[dt][:], cb)

            # Stage 1: U[(h%2)*64+dv, (pair, q)] = sum_din Wv[h,dv,din]*ga
            ups = u_pool.tile([128, N_PAIR * NQ], f32, tag="u", name=f"u_{b}")
            for dt in range(N_DT):
                for h in range(H):
                    pair, hl = h // 2, h % 2
                    nc.tensor.matmul(
                        ups[hl * 64:(hl + 1) * 64,
                            pair * NQ:(pair + 1) * NQ],
                        wv_sb[:, (dt * H + h) * D_V:(dt * H + h + 1) * D_V],
                        ga[:, dt * HQ + h * NQ: dt * HQ + (h + 1) * NQ],
                        start=(dt == 0 and h < 2), stop=(dt == N_DT - 1),
                        skip_group_check=True)
            usb = usb_pool.tile([128, N_PAIR * NQ], bf16, tag="usb",
                                name=f"usb_{b}")
            nc.vector.tensor_mul(usb[:], ups[:], zbu[:])

            # Stage 2: out[q, dm] = sum_{pair} U_pair.T-contraction with Wo
            ops = z_pool.tile([128, D_MODEL], f32, tag="z", name=f"o_{b}")
            for pair in range(N_PAIR):
                nc.tensor.matmul(
                    ops[0:NQ, :],
                    usb[:, pair * NQ:(pair + 1) * NQ],
                    wo_sb[:, pair * D_MODEL:(pair + 1) * D_MODEL],
                    start=(pair == 0), stop=False)
            nc.tensor.matmul(ops[0:NQ, :], ones_sb[0:1, 0:NQ], beff_sb[:],
                             start=False, stop=True)
            out_sb = usb_pool.tile([NQ, D_MODEL], f32, tag="osb",
                                   name=f"osb_{b}")
            # split copy+DMA so the first half's DMA overlaps the second copy
            nc.vector.tensor_copy(out_sb[0:NQ // 2, :], ops[0:NQ // 2, :])
            nc.sync.dma_start(out.ap()[b * NQ:b * NQ + NQ // 2, :],
                              out_sb[0:NQ // 2, :])
            nc.vector.tensor_copy(out_sb[NQ // 2:NQ, :],
                                  ops[NQ // 2:NQ, :])
            nc.sync.dma_start(out.ap()[b * NQ + NQ // 2:(b + 1) * NQ, :],
                              out_sb[NQ // 2:NQ, :])

    nc.compile()
    return nc


def _get_nc():
    if "nc" not in _NC_CACHE:
        _NC_CACHE["nc"] = _build_nc()
    return _NC_CACHE["nc"]


def _host_prep(att12, att3, values, W_v, b_v, W_o, b_o):
    att12 = np.asarray(att12, np.float32)
    att3 = np.asarray(att3, np.float32)
    values = np.asarray(values, np.float32)
    W_v = np.asarray(W_v, np.float32)
    b_v = np.asarray(b_v, np.float32)
    W_o = np.asarray(W_o, np.float32)
    b_o = np.asarray(b_o, np.float32)

    # k-tiles 0-3 stay bf16 (0-7 for each core's local batch 0, as PE
    # runway during the clock ramp); the rest go fp8 DoubleRow with an
    # expm1 shift and exact fp32 ones-correction c_all = sum over the
    # fp8-path keys of values (per batch, since the split differs)
    values_bf = np.ascontiguousarray(
        values[:, _PERM[:8 * 128], :].astype(BF16)
        .reshape(B, 8, 128, D_IN).transpose(0, 2, 1, 3)
        .reshape(B, 128, 8 * D_IN))
    v8 = values.astype(FP8)
    idx = _PERM[4 * 128:].reshape(14, 2, 128)            # [pp, i, p]
    values_dr = np.ascontiguousarray(
        v8[:, idx, :].transpose(0, 1, 3, 2, 4)           # [b, pp, p, i, din]
        .reshape(B, 7, 2, 128, 2 * D_IN)
        .transpose(0, 1, 3, 2, 4)
        .reshape(B, 7, 128, 4 * D_IN))
    nbf = np.where(np.arange(B) % B_LOC == 0, 8, 4)
    c_full = np.stack([values[i, _PERM[nbf[i] * 128:], :].sum(axis=0)
                       for i in range(B)])               # [B, 512] fp32
    c_all = np.ascontiguousarray(
        c_full.reshape(B, N_DT, 128).transpose(0, 2, 1)) # [b, p, dt]
    att3_t = np.ascontiguousarray(
        att3.transpose(0, 3, 1, 2).reshape(B, NCELL, HQ)).astype(BF16)
    att12_r = np.ascontiguousarray(
        att12.transpose(0, 1, 2, 4, 5, 3).reshape(B, NCELL, F * H)).astype(BF16)
    a12_pair = np.broadcast_to(
        att12_r[:, :, :, None], (B, NCELL, F * H, 2)).reshape(B, NCELL, A12W)
    # Z-poly coefficients: least-squares quadratic fit of g(a) = sum_f
    # exp(a*a12_f) over a in [0,1], per (b, c, h). Stored T2,T1,T0,
    # pre-expanded over q so the kernel reads plain packed APs.
    # att12 is [B, CH, CW, H, FH, FW] -> [b, c=(ch,cw), h, f=(fh,fw)]
    a12hc = att12.reshape(B, NCELL, H, F).astype(np.float64)
    grid = np.linspace(0.0, 1.0, 17)
    vand = np.vander(grid, NJ, increasing=True)          # [17, NJ]
    pinv = np.linalg.pinv(vand)                          # [NJ, 17]
    gvals = np.exp(grid[:, None, None, None, None] * a12hc[None]).sum(-1)
    coef = np.einsum('jg,gbch->jbch', pinv, gvals).astype(np.float32)
    Ts = []
    for j in range(NJ - 1, -1, -1):
        Ts.append(np.broadcast_to(coef[j][:, :, :, None],
                                  (B, NCELL, H, NQ)).reshape(B, NCELL, HQ))
    att12_pt = np.ascontiguousarray(
        np.concatenate([a12_pair] + Ts, axis=2)).astype(BF16)

    # wv_all[p, (dt, h, dv)] = W_v[h*D_V+dv, dt*128+p]
    Wv3 = W_v.reshape(H, D_V, N_DT, 128)              # [h, dv, dt, p]
    wv_all = np.ascontiguousarray(
        Wv3.transpose(3, 2, 0, 1).reshape(128, N_DT * H * D_V)).astype(BF16)
    # wo_all[p=(hl*64+dv), (pair, dm)] = W_o[dm, (pair*2+hl)*64+dv]
    Wo4 = W_o.reshape(D_MODEL, N_PAIR, 2, D_V)        # [dm, pair, hl, dv]
    wo_all = np.ascontiguousarray(
        Wo4.transpose(2, 3, 1, 0).reshape(128, N_PAIR * D_MODEL)).astype(BF16)

    b_eff = b_o + W_o @ b_v
    beff = b_eff.reshape(1, D_MODEL).astype(BF16)
    return {"values_dr": values_dr, "values_bf": values_bf, "c_all": c_all,
            "att3_t": att3_t, "att12_pt": att12_pt,
            "wv_all": wv_all, "wo_all": wo_all, "beff": beff}


def kernel(att12, att3, values, W_v, b_v, W_o, b_o):
    from concourse.bass_utils import run_bass_kernel_spmd

    ins = _host_prep(att12, att3, values, W_v, b_v, W_o, b_o)

    in_maps = []
    for core in range(N_CORES):
        s = slice(core * B_LOC, (core + 1) * B_LOC)
        in_maps.append({k: (np.ascontiguousarray(v[s]) if v.shape[0] == B
                            else v)
                        for k, v in ins.items()})

    nc = _get_nc()
    res = run_bass_kernel_spmd(nc, in_maps, core_ids=list(range(N_CORES)))
    out = np.concatenate(
        [res.results[i]["out"].reshape(B_LOC, NQ, D_MODEL)
         for i in range(N_CORES)], axis=0)
    return out.astype(np.float32)


# revision 50
# speedup vs baseline: 1.0240x; 1.0240x over previous
"""Trainium2 Bass kernel for nn_BoostEnhancedAttention.

Reference computation:
    v   = (values @ W_v.T + b_v)                      # [B, NK, H*D_V]
    att = softmax(att3 * att12 interleaved, axis=k)   # [B, H, NQ, NK]
    out = (att @ v_per_head) @ W_o.T + b_o            # [B, NQ, D_MODEL]

Restructuring used here (verified vs reference, rel err ~1.2e-2):
  - Scores factor as s[b,h,q,k] = att3[b,h,q,c(k)] * att12[b,h,...f(k)];
    E = exp(s) built by DVE broadcast-multiply + ACT exp.
  - Attention applied BEFORE the projections: G[d_in,(h,q)] =
    sum_k values[k,d_in] * (E[k,(h,q)]-1) via fp8 e4m3 DoubleRow matmuls
    (ALL 32 k-tiles), plus exact fp32 ones-correction C = sum_k values
    folded into the ga-copy bias. expm1 shift halves fp8 quant error.
  - Softmax denominator Z by 4th-order Taylor: Z[h,q] =
    sum_c sum_j T_j[h,c]*a3[q,c]^j, T_j = sum_f a12^j/j! (host),
    evaluated by a short Horner chain on DVE + ones-matmuls. This
    replaces the per-group E-summation entirely (saves ~4.5us DVE per
    batch and removes the exp->sum dependency chain).
  - Projections after normalize, per head: U = ga @ W_v_h.T, then
    out = U.T-contraction with W_o + bias via K=1 matmul.
  - fp8 converts (E-1 -> e4m3) round-robined over DVE/ACT/Pool so no
    single elementwise engine bottlenecks; exp stays the ACT floor.

Sharding: data-parallel over batch, B=32 over 8 cores -> 4 batches/core.
No collectives needed; outputs concatenated on host.
"""

import numpy as np
import ml_dtypes

B, CH, CW, H, FH, FW = 32, 16, 16, 8, 4, 4
NQ = 64
NCELL = CH * CW          # 256 coarse cells (c)
F = FH * FW              # 16 fine positions per cell
NK = NCELL * F           # 4096
D_IN, D_V, D_MODEL = 512, 64, 512
N_CORES = 8
B_LOC = B // N_CORES     # 4
N_DT = 4                 # d_in tiles of 128
HQ = H * NQ              # 512
N_PAIR = H // 2          # head-pairs for the U projection tiles
NJ = 3                   # fitted quadratic for Z: coeffs T2..T0 per (c,h)
A12W = F * H * 2         # att12 pair-dup block width
TW = NJ * HQ             # T coeffs pre-expanded over q (packed DVE 2x APs)

BF16 = ml_dtypes.bfloat16
FP8 = ml_dtypes.float8_e4m3


def _k_perm():
    """perm[k'] -> original k, where k' = (half*16+f)*128 + c_loc."""
    perm = np.zeros(NK, np.int64)
    c = np.arange(NCELL)
    ch_i, cw_i = c // CW, c % CW
    for half in range(2):
        for f in range(F):
            kt = half * F + f
            fh, fw = f // FW, f % FW
            cc = half * 128 + np.arange(128)
            perm[kt * 128:(kt + 1) * 128] = (
                ch_i[cc] * (FH * CW * FW) + fh * (CW * FW) + cw_i[cc] * FW + fw
            )
    return perm


_PERM = _k_perm()
_NC_CACHE = {}

# per-batch group schedule: (half, f0, FQ); b0 splits half0's first tiles.
# group (half=0, f0<4) stays bf16 (k-tiles 0-3): no fp8 convert, and the
# extra PE time keeps the HAM clock gate from throttling the ramp.
_ORDER_B0 = [(0, 0, 1), (0, 1, 1), (0, 2, 2), (0, 4, 4), (1, 0, 4),
             (0, 8, 4), (1, 4, 4), (0, 12, 4), (1, 8, 4), (1, 12, 4)]
_ORDER = [(0, 0, 4), (0, 4, 4), (1, 0, 4), (0, 8, 4),
          (1, 4, 4), (0, 12, 4), (1, 8, 4), (1, 12, 4)]
# engine for the fp8 convert of each fp8 group in order: D=DVE, A=ACT.
# A-convs serialize behind exps on the ACT queue, so keep them few and
# mid-batch (early ones stall the whole exp stream, late ones gate the
# last AV groups).
_CONV = "DDADADD"
# engine for the 4 ga copies (psum->sbuf + ones-correction bias)
_GA = "ADAD"


def _build_nc():
    from contextlib import ExitStack

    import concourse.bass as bass
    import concourse.tile as tile
    from concourse import bacc, mybir

    f32 = mybir.dt.float32
    bf16 = mybir.dt.bfloat16
    fp8 = mybir.dt.float8e4

    nc = bacc.Bacc("TRN2", target_bir_lowering=False, debug=False,
                   num_devices=N_CORES)

    values_dr = nc.dram_tensor("values_dr", [B_LOC, 7, 128, 4 * D_IN],
                               fp8, kind="ExternalInput")
    values_bf = nc.dram_tensor("values_bf", [B_LOC, 128, 8 * D_IN],
                               bf16, kind="ExternalInput")
    c_all = nc.dram_tensor("c_all", [B_LOC, 128, N_DT], f32,
                           kind="ExternalInput")
    att3_t = nc.dram_tensor("att3_t", [B_LOC, NCELL, HQ], bf16,
                            kind="ExternalInput")
    att12_pt = nc.dram_tensor("att12_pt", [B_LOC, NCELL, A12W + TW], bf16,
                              kind="ExternalInput")
    wv_all = nc.dram_tensor("wv_all", [128, N_DT * H * D_V], bf16,
                            kind="ExternalInput")
    wo_all = nc.dram_tensor("wo_all", [128, N_PAIR * D_MODEL], bf16,
                            kind="ExternalInput")
    beff = nc.dram_tensor("beff", [1, D_MODEL], bf16, kind="ExternalInput")
    out = nc.dram_tensor("out", [B_LOC * NQ, D_MODEL], f32,
                         kind="ExternalOutput")

    with tile.TileContext(nc) as tc, ExitStack() as ctx:
        const_pool = ctx.enter_context(tc.tile_pool(name="const", bufs=1))
        a3_pool = ctx.enter_context(tc.tile_pool(name="a3", bufs=2))
        a12r_pool = ctx.enter_context(tc.tile_pool(name="a12r", bufs=2))
        sc_pool = ctx.enter_context(tc.tile_pool(name="sc", bufs=6))
        et_pool = ctx.enter_context(tc.tile_pool(name="et", bufs=6))
        et8_pool = ctx.enter_context(tc.tile_pool(name="et8", bufs=7))
        vdr_pool = ctx.enter_context(tc.tile_pool(name="vdr", bufs=8))
        vbf_pool = ctx.enter_context(tc.tile_pool(name="vbf", bufs=2))
        p_pool = ctx.enter_context(tc.tile_pool(name="pp", bufs=2))
        zb_pool = ctx.enter_context(tc.tile_pool(name="zb", bufs=2))
        ga_pool = ctx.enter_context(tc.tile_pool(name="ga", bufs=2))
        usb_pool = ctx.enter_context(tc.tile_pool(name="usb", bufs=2))
        g_pool = ctx.enter_context(tc.tile_pool(name="gps", bufs=1, space="PSUM"))
        u_pool = ctx.enter_context(tc.tile_pool(name="ups", bufs=1, space="PSUM"))
        z_pool = ctx.enter_context(tc.tile_pool(name="zps", bufs=1, space="PSUM"))

        Q2 = NQ // 2

        def prologue(b, first=False):
            """Input DMAs + first score group for batch b."""
            a3_t = [a3_pool.tile([128, HQ], bf16, tag=f"a3_{hf}",
                                 name=f"a3_{b}_{hf}") for hf in range(2)]
            a12r_t = [a12r_pool.tile([128, A12W + TW], bf16, tag=f"a12r_{hf}",
                                     name=f"a12r_{b}_{hf}") for hf in range(2)]
            vbf = vbf_pool.tile([128, 8 * D_IN], bf16, tag="vbf",
                                name=f"vbf_{b}")
            if first:
                # startup: interleave so the critical chain (a3/a12 hf0 ->
                # mul -> exp, bf16 values for the first matmuls) is
                # dispatched first on the serial sync queue. b0 runs TWO
                # bf16 groups (8 k-tiles) as PE runway while the clock and
                # the score pipeline ramp up.
                nc.sync.dma_start(a3_t[0][:], att3_t.ap()[b, 0:128, :])
                nc.sync.dma_start(a12r_t[0][:], att12_pt.ap()[b, 0:128, :])
                nc.sync.dma_start(vbf[:], values_bf.ap()[b])
                nc.sync.dma_start(a3_t[1][:], att3_t.ap()[b, 128:256, :])
                nc.sync.dma_start(a12r_t[1][:], att12_pt.ap()[b, 128:256, :])
                vdr_pre = None
            else:
                for hf in range(2):
                    nc.sync.dma_start(a3_t[hf][:],
                                      att3_t.ap()[b, hf * 128:(hf + 1) * 128, :])
                    nc.sync.dma_start(a12r_t[hf][:],
                                      att12_pt.ap()[b, hf * 128:(hf + 1) * 128, :])
                nc.sync.dma_start(vbf[:, :4 * D_IN],
                                  values_bf.ap()[b][:, :4 * D_IN])
                vdr_pre = vdr_pool.tile([128, 4 * D_IN], fp8, tag="vdr",
                                        name=f"vdr_{b}_0")
                nc.sync.dma_start(vdr_pre[:], values_dr.ap()[b, 0])
            order = _ORDER_B0 if b == 0 else _ORDER
            # emit group 0 (mul+exp) ahead of the previous epilogue
            et8s = {0: emit_group(b, order[0], None, a3_t, a12r_t)}
            return a3_t, a12r_t, vbf, order, et8s, vdr_pre

        def emit_mul(b, grp, a3_t, a12r_t):
            """Broadcast multiply for one score group -> sc tile."""
            half, f0, FQ = grp
            a3b = a3_t[half][:]
            in0 = bass.AP(a3b.tensor, a3b.offset,
                          [a3b.ap[0], [0, FQ], [NQ, H], [2, Q2], [1, 2]])
            sc = sc_pool.tile([128, 4 * HQ], bf16, tag="sc",
                              name=f"sc_{b}_{half}_{f0}")
            scb = sc[:]
            out_ap = bass.AP(scb.tensor, scb.offset,
                             [scb.ap[0], [HQ, FQ], [NQ, H], [2, Q2], [1, 2]])
            a12b = a12r_t[half][:]
            in1 = bass.AP(a12b.tensor, a12b.offset + f0 * H * 2,
                          [a12b.ap[0], [H * 2, FQ], [2, H], [0, Q2], [1, 2]])
            nc.vector.tensor_mul(out_ap, in0, in1)
            return sc

        def emit_group(b, grp, conv_eng, a3_t, a12r_t, sc=None):
            """One score group: broadcast multiply, exp, fp8 convert.
            conv_eng None -> bf16 group (no convert), returns et."""
            half, f0, FQ = grp
            if sc is None:
                sc = emit_mul(b, grp, a3_t, a12r_t)
            et = et_pool.tile([128, 4 * HQ], bf16, tag="et",
                              name=f"et_{b}_{half}_{f0}")
            nc.scalar.activation(et[:, :FQ * HQ], sc[:, :FQ * HQ],
                                 mybir.ActivationFunctionType.Exp)
            if conv_eng is None:
                return et
            et8 = et8_pool.tile([128, 4 * HQ], fp8, tag="et8",
                                name=f"et8_{b}_{half}_{f0}")
            n = FQ * HQ
            if conv_eng == "D":
                nc.vector.tensor_scalar_sub(et8[:, :n], et[:, :n], 1.0)
            else:
                nc.scalar.activation(et8[:, :n], et[:, :n],
                                     mybir.ActivationFunctionType.Identity,
                                     bias=neg1_sb[:])
            return et8

        def emit_poly(b, half, a3_t, a12r_t):
            """Z-poly for one half: P = (T2*a + T1)*a + T0, least-squares
            fitted quadratic coefficients (host). NOTE: keep off the Pool
            engine -- GpSimd shares SBUF ports with the DVE and slows every
            concurrent DVE op by ~1.65x (measured)."""
            P = p_pool.tile([128, HQ], bf16, tag=f"p_{half}",
                            name=f"p_{b}_{half}")
            a3b = a3_t[half][:]
            a12b = a12r_t[half][:]
            Pb = P[:]

            def tb(jj):
                # T coeffs pre-expanded over q on host: plain packed APs
                # keep the DVE in its 2x mode (broadcast APs drop to 1x)
                return bass.AP(a12b.tensor, a12b.offset + A12W + jj * HQ,
                               [a12b.ap[0], [1, HQ]])

            nc.vector.tensor_mul(Pb, a3b, tb(0))
            nc.vector.tensor_add(Pb, Pb, tb(1))
            nc.vector.tensor_mul(Pb, Pb, a3b)
            nc.vector.tensor_add(Pb, Pb, tb(2))
            return P

        ones_sb = const_pool.tile([128, 128], bf16)
        nc.vector.memset(ones_sb[:], 1.0)
        warm_sb = const_pool.tile([128, D_MODEL], bf16, name="warm_sb")
        nc.vector.memset(warm_sb[:], 1.0)
        # warm matmuls bridge engine boot -> first attention matmul so the
        # HAM clock gate ramps the PE before real work arrives
        warm = z_pool.tile([128, HQ], f32, tag="z", name="warm")
        for wi in range(12):
            nc.tensor.matmul(warm[:], ones_sb[:], warm_sb[:],
                             start=True, stop=True)
        beff_sb = const_pool.tile([1, D_MODEL], bf16)
        neg1_sb = const_pool.tile([128, 1], f32, name="neg1")
        nc.vector.memset(neg1_sb[:], -1.0)
        # dummy ACT op at t=0 hoists the exp table load off the critical path
        actwarm = const_pool.tile([128, 1], bf16, name="actwarm")
        nc.scalar.activation(actwarm[:], ones_sb[:, 0:1],
                             mybir.ActivationFunctionType.Exp)
        c_sb = const_pool.tile([128, B_LOC * N_DT], f32, name="c_sb")

        pro = prologue(0, first=True)

        for b in range(B_LOC):
            a3_t, a12r_t, vbf, order, et8s, vdr_pre = pro
            vdr_t = {} if vdr_pre is None else {0: vdr_pre}
            nbf = 8 if b == 0 else 4
            gps = [g_pool.tile([128, HQ], f32, tag=f"g{dt}", name=f"g_{b}_{dt}",
                               bufs=(2 if dt < 2 else 1))
                   for dt in range(N_DT)]
            started = [False] * N_DT
            n_grp = len(order)
            P_t = [None, None]
            fp8_idx = 0

            for gi, grp in enumerate(order):
                half, f0, FQ = grp
                is_bf = (half == 0 and f0 < nbf)
                if gi in et8s:
                    et8 = et8s[gi]
                elif is_bf:
                    et8 = emit_group(b, grp, None, a3_t, a12r_t)
                else:
                    et8 = emit_group(b, grp, _CONV[fp8_idx], a3_t, a12r_t)
                if not is_bf:
                    fp8_idx += 1
                last_grp = (gi == n_grp - 1)
                eb = et8[:]
                if is_bf:
                    # bf16 AV: plain matmuls straight off the exp output
                    vb = vbf[:]
                    for j in range(FQ):
                        kt = f0 + j
                        for dt in range(N_DT):
                            nc.tensor.matmul(
                                gps[dt][:],
                                vbf[:, kt * D_IN + dt * 128:
                                    kt * D_IN + (dt + 1) * 128],
                                et8[:, j * HQ:(j + 1) * HQ],
                                start=not started[dt], stop=False)
                            started[dt] = True
                else:
                    vg = 4 * half + f0 // 4 - 1
                    if vg not in vdr_t:
                        vdr_t[vg] = vdr_pool.tile([128, 4 * D_IN], fp8,
                                                  tag="vdr",
                                                  name=f"vdr_{b}_{vg}")
                        nc.sync.dma_start(vdr_t[vg][:], values_dr.ap()[b, vg])
                    vdr = vdr_t[vg]
                    vb = vdr[:]
                    for pidx in range(2):
                        last_p = last_grp and pidx == 1
                        for dt in range(N_DT):
                            lhsT = bass.AP(vb.tensor,
                                           vb.offset + pidx * 2 * D_IN
                                           + dt * 128,
                                           [vb.ap[0], [D_IN, 2], [1, 128]])
                            rhs = bass.AP(eb.tensor, eb.offset + pidx * 2 * HQ,
                                          [eb.ap[0], [HQ, 2], [1, HQ]])
                            nc.tensor.matmul(
                                gps[dt][:], lhsT, rhs,
                                start=not started[dt], stop=last_p,
                                perf_mode=mybir.MatmulPerfMode.DoubleRow)
                            started[dt] = True

                # Z-poly per half on the DVE, early in the batch
                if gi == 0:
                    P_t[0] = emit_poly(b, 0, a3_t, a12r_t)
                elif gi == 1:
                    P_t[1] = emit_poly(b, 1, a3_t, a12r_t)
                if b == 0 and gi == 5:
                    # epilogue-only constants after the critical prefetches
                    nc.sync.dma_start(beff_sb[:], beff.ap())
                    for cb in range(B_LOC):
                        nc.sync.dma_start(c_sb[:, cb * N_DT:(cb + 1) * N_DT],
                                          c_all.ap()[cb])
                if b == 0 and gi == 6:
                    # projection weights mid-stream: transfer never contends
                    # with critical prefetches
                    wv_sb = const_pool.tile([128, N_DT * H * D_V], bf16,
                                            name="wv_sb")
                    nc.sync.dma_start(wv_sb[:], wv_all.ap())
                    wo_sb = const_pool.tile([128, N_PAIR * D_MODEL], bf16,
                                            name="wo_sb")
                    nc.sync.dma_start(wo_sb[:], wo_all.ap())

            if b + 1 < B_LOC:
                pro = prologue(b + 1)

            # Z = column sums of P via two ones-matmuls per half; output
            # partitions 0-63 get even-head sums, 64-127 odd heads
            zps = z_pool.tile([128, 2 * N_PAIR * NQ], f32, tag="z",
                              name=f"z_{b}")
            for hf in range(2):
                pb = P_t[hf][:]
                for hl in range(2):
                    mov = bass.AP(pb.tensor, pb.offset + hl * NQ,
                                  [pb.ap[0], [2 * NQ, N_PAIR], [1, NQ]])
                    nc.tensor.matmul(
                        zps[hl * 64:(hl + 1) * 64, 0:N_PAIR * NQ],
                        ones_sb[:, hl * 64:(hl + 1) * 64], mov,
                        start=(hf == 0), stop=(hf == 1))
            zbu = zb_pool.tile([128, N_PAIR * NQ], f32)
            nc.vector.reciprocal_approx_fast(zbu[:], zps[:, 0:N_PAIR * NQ])

            # unnormalized attention output to sbuf; bias adds the exact
            # fp32 sum of all values rows (the "+1" of each expm1 key)
            ga = ga_pool.tile([128, N_DT * HQ], bf16, tag="ga", name=f"ga_{b}")
            for dt in range(N_DT):
                cb = c_sb[:, b * N_DT + dt:b * N_DT + dt + 1]
                if _GA[dt] == "A":
                    nc.scalar.activation(
                        ga[:, dt * HQ:(dt + 1) * HQ], gps[dt][:],
                        mybir.ActivationFunctionType.Identity, bias=cb)
                else:
                    nc.vector.tensor_scalar_add(
                        ga[:, dt * HQ:(dt + 1) * HQ], gps[dt][:], cb)

            # Stage 1: U[(h%2)*64+dv, (pair, q)] = sum_din Wv[h,dv,din]*ga
            ups = u_pool.tile([128, N_PAIR * NQ], f32, tag="u", name=f"u_{b}")
            for dt in range(N_DT):
                for h in range(H):
                    pair, hl = h // 2, h % 2
                    nc.tensor.matmul(
                        ups[hl * 64:(hl + 1) * 64,
                            pair * NQ:(pair + 1) * NQ],
                        wv_sb[:, (dt * H + h) * D_V:(dt * H + h + 1) * D_V],
                        ga[:, dt * HQ + h * NQ: dt * HQ + (h + 1) * NQ],
                        start=(dt == 0 and h < 2), stop=(dt == N_DT - 1),
                        skip_group_check=True)
            usb = usb_pool.tile([128, N_PAIR * NQ], bf16, tag="usb",
                                name=f"usb_{b}")
            nc.vector.tensor_mul(usb[:], ups[:], zbu[:])

            # Stage 2: out[q, dm] = sum_{pair} U_pair.T-contraction with Wo
            ops = z_pool.tile([128, D_MODEL], f32, tag="z", name=f"o_{b}")
            for pair in range(N_PAIR):
                nc.tensor.matmul(
                    ops[0:NQ, :],
                    usb[:, pair * NQ:(pair + 1) * NQ],
                    wo_sb[:, pair * D_MODEL:(pair + 1) * D_MODEL],
                    start=(pair == 0), stop=False)
            nc.tensor.matmul(ops[0:NQ, :], ones_sb[0:1, 0:NQ], beff_sb[:],
                             start=False, stop=True)
            out_sb = usb_pool.tile([NQ, D_MODEL], f32, tag="osb",
                                   name=f"osb_{b}")
            nc.vector.tensor_copy(out_sb[:], ops[0:NQ, :])
            nc.sync.dma_start(out.ap()[b * NQ:(b + 1) * NQ, :], out_sb[:])

    nc.compile()
    return nc


def _get_nc():
    if "nc" not in _NC_CACHE:
        _NC_CACHE["nc"] = _build_nc()
    return _NC_CACHE["nc"]


def _host_prep(att12, att3, values, W_v, b_v, W_o, b_o):
    att12 = np.asarray(att12, np.float32)
    att3 = np.asarray(att3, np.float32)
    values = np.asarray(values, np.float32)
    W_v = np.asarray(W_v, np.float32)
    b_v = np.asarray(b_v, np.float32)
    W_o = np.asarray(W_o, np.float32)
    b_o = np.asarray(b_o, np.float32)

    # k-tiles 0-3 stay bf16 (0-7 for each core's local batch 0, as PE
    # runway during the clock ramp); the rest go fp8 DoubleRow with an
    # expm1 shift and exact fp32 ones-correction c_all = sum over the
    # fp8-path keys of values (per batch, since the split differs)
    values_bf = np.ascontiguousarray(
        values[:, _PERM[:8 * 128], :].astype(BF16)
        .reshape(B, 8, 128, D_IN).transpose(0, 2, 1, 3)
        .reshape(B, 128, 8 * D_IN))
    v8 = values.astype(FP8)
    idx = _PERM[4 * 128:].reshape(14, 2, 128)            # [pp, i, p]
    values_dr = np.ascontiguousarray(
        v8[:, idx, :].transpose(0, 1, 3, 2, 4)           # [b, pp, p, i, din]
        .reshape(B, 7, 2, 128, 2 * D_IN)
        .transpose(0, 1, 3, 2, 4)
        .reshape(B, 7, 128, 4 * D_IN))
    nbf = np.where(np.arange(B) % B_LOC == 0, 8, 4)
    c_full = np.stack([values[i, _PERM[nbf[i] * 128:], :].sum(axis=0)
                       for i in range(B)])               # [B, 512] fp32
    c_all = np.ascontiguousarray(
        c_full.reshape(B, N_DT, 128).transpose(0, 2, 1)) # [b, p, dt]
    att3_t = np.ascontiguousarray(
        att3.transpose(0, 3, 1, 2).reshape(B, NCELL, HQ)).astype(BF16)
    att12_r = np.ascontiguousarray(
        att12.transpose(0, 1, 2, 4, 5, 3).reshape(B, NCELL, F * H)).astype(BF16)
    a12_pair = np.broadcast_to(
        att12_r[:, :, :, None], (B, NCELL, F * H, 2)).reshape(B, NCELL, A12W)
    # Z-poly coefficients: least-squares quadratic fit of g(a) = sum_f
    # exp(a*a12_f) over a in [0,1], per (b, c, h). Stored T2,T1,T0,
    # pre-expanded over q so the kernel reads plain packed APs.
    # att12 is [B, CH, CW, H, FH, FW] -> [b, c=(ch,cw), h, f=(fh,fw)]
    a12hc = att12.reshape(B, NCELL, H, F).astype(np.float64)
    grid = np.linspace(0.0, 1.0, 17)
    vand = np.vander(grid, NJ, increasing=True)          # [17, NJ]
    pinv = np.linalg.pinv(vand)                          # [NJ, 17]
    gvals = np.exp(grid[:, None, None, None, None] * a12hc[None]).sum(-1)
    coef = np.einsum('jg,gbch->jbch', pinv, gvals).astype(np.float32)
    Ts = []
    for j in range(NJ - 1, -1, -1):
        Ts.append(np.broadcast_to(coef[j][:, :, :, None],
                                  (B, NCELL, H, NQ)).reshape(B, NCELL, HQ))
    att12_pt = np.ascontiguousarray(
        np.concatenate([a12_pair] + Ts, axis=2)).astype(BF16)

    # wv_all[p, (dt, h, dv)] = W_v[h*D_V+dv, dt*128+p]
    Wv3 = W_v.reshape(H, D_V, N_DT, 128)              # [h, dv, dt, p]
    wv_all = np.ascontiguousarray(
        Wv3.transpose(3, 2, 0, 1).reshape(128, N_DT * H * D_V)).astype(BF16)
    # wo_all[p=(hl*64+dv), (pair, dm)] = W_o[dm, (pair*2+hl)*64+dv]
    Wo4 = W_o.reshape(D_MODEL, N_PAIR, 2, D_V)        # [dm, pair, hl, dv]
    wo_all = np.ascontiguousarray(
        Wo4.transpose(2, 3, 1, 0).reshape(128, N_PAIR * D_MODEL)).astype(BF16)

    b_eff = b_o + W_o @ b_v
    beff = b_eff.reshape(1, D_MODEL).astype(BF16)
    return {"values_dr": values_dr, "values_bf": values_bf, "c_all": c_all,
            "att3_t": att3_t, "att12_pt": att12_pt,
            "wv_all": wv_all, "wo_all": wo_all, "beff": beff}


def kernel(att12, att3, values, W_v, b_v, W_o, b_o):
    from concourse.bass_utils import run_bass_kernel_spmd

    ins = _host_prep(att12, att3, values, W_v, b_v, W_o, b_o)

    in_maps = []
    for core in range(N_CORES):
        s = slice(core * B_LOC, (core + 1) * B_LOC)
        in_maps.append({k: (np.ascontiguousarray(v[s]) if v.shape[0] == B
                            else v)
                        for k, v in ins.items()})

    nc = _get_nc()
    res = run_bass_kernel_spmd(nc, in_maps, core_ids=list(range(N_CORES)))
    out = np.concatenate(
        [res.results[i]["out"].reshape(B_LOC, NQ, D_MODEL)
         for i in range(N_CORES)], axis=0)
    return out.astype(np.float32)
